# revision 15
# baseline (speedup 1.0000x reference)
"""Trainium2 Bass kernel: row-parallel linear  y = einsum('sbk,nk->sbn', x, W) + bias.

Strategy (default variant v5)
-----------------------------
Full inputs arrive on the host. We flatten (seq, batch) -> M = 8192 rows and
shard M across the 8 NeuronCores (1024 rows each); every core streams the full
weight. Each core computes its [1024, 4096] output slice as a single-pass bf16
GEMM accumulated in fp32 PSUM: operand rounding to bf16 yields ~8e-4 max rel
err on this K=16384 randn problem, far inside the 2e-2 gate, so no hi/lo
correction passes are needed (they were 3x the PE work).

Device layout: operands are staged in DRAM as [p=128, ko=K/128, m] with
k = ko*128 + p, so every SBUF tile load is a contiguous-per-partition DMA and
the contraction dim lands on the partition axis, as the PE array requires.

Per core: 2 m-blocks of 512 rows; the m-block's full-K x strip lives in SBUF
as 16 chunk tiles in a 18-deep pool so the next block's x prefetches under the
current block's last phase; W streams twice (once per m-block) in 1 MB
[128, 4ko, 1024] slabs alternating across the two HWDGE rings (sync/scalar);
8 PSUM banks hold the 4x2 (m-strip x n-tile) fp32 accumulators across the
whole K loop and are evicted once per (m-block, n-chunk) phase via VectorE,
with output stores on the gpsimd (SWDGE) ring so they never block the W
stream. Older variants (v1 3-pass split-bf16, v2 K-blocked, v3 per-ko W
tiles, v4 monolithic x strip) are kept for A/B via KERNEL_VARIANT.
"""

import os

import numpy as np
import ml_dtypes

BF16 = ml_dtypes.bfloat16

# Problem shapes (hardcoded per contest contract).
SEQ, BATCH, D_FF, D_MODEL = 2048, 4, 16384, 4096
N_CORES = 8
P = 128

M_FULL = SEQ * BATCH            # 8192
M_CORE = M_FULL // N_CORES      # 1024

# Tiling parameters.
M_BLOCK = 256                   # x strip width kept resident in SBUF
N_CHUNK = 2048                  # streamed W chunk width
MM_N = 512                      # matmul free dim (one fp32 PSUM bank)

# Exec-time of the last hardware benchmark (ns), populated when KERNEL_BENCH>0.
LAST_EXEC_NS = None
LAST_RESULTS = None

_BUILD_CACHE = {}
_RUNNER_CACHE = {}


def _build_nc(k, m_core, n, m_block=M_BLOCK, n_chunk=N_CHUNK, reps=1):
    """Build + compile the per-core Bass module for a [m_core, k] x [n, k]^T GEMM.

    reps>1 repeats the whole GEMM inside the program (benchmark variants; the
    timing difference between reps=K and reps=1 isolates steady-state kernel
    time from dispatch overhead)."""
    import concourse.mybir as mybir
    import concourse.tile as tile
    from concourse import bacc

    ko_n = k // P               # number of 128-row k chunks
    n_mb = m_core // m_block
    n_nc = n // n_chunk
    ms_n = m_block // P         # m strips per block
    nt_n = n_chunk // MM_N      # n tiles per chunk
    assert ms_n * nt_n <= 8, "PSUM banks exceeded"

    nc = bacc.Bacc(None, target_bir_lowering=False, debug=False)
    xhi = nc.declare_dram_parameter("xhi", [P, ko_n, m_core], mybir.dt.bfloat16, isOutput=False)
    xlo = nc.declare_dram_parameter("xlo", [P, ko_n, m_core], mybir.dt.bfloat16, isOutput=False)
    whi = nc.declare_dram_parameter("whi", [P, ko_n, n], mybir.dt.bfloat16, isOutput=False)
    wlo = nc.declare_dram_parameter("wlo", [P, ko_n, n], mybir.dt.bfloat16, isOutput=False)
    out = nc.declare_dram_parameter("out", [m_core, n], mybir.dt.float32, isOutput=True)

    f32 = mybir.dt.float32
    bf16 = mybir.dt.bfloat16

    with tile.TileContext(nc) as tc:
        with (
            tc.tile_pool(name="xpool", bufs=1) as xpool,
            tc.tile_pool(name="wpool", bufs=6) as wpool,
            tc.tile_pool(name="opool", bufs=4) as opool,
            tc.tile_pool(name="pspool", bufs=8, space="PSUM") as pspool,
        ):
            for rep, mb in ((r_, m_) for r_ in range(reps) for m_ in range(n_mb)):
                m0 = mb * m_block
                # Resident x strips for this m-block: [P, ko_n, m_block] hi/lo.
                xh = xpool.tile([P, ko_n, m_block], bf16, tag="xh")
                xl = xpool.tile([P, ko_n, m_block], bf16, tag="xl")
                # Load in ko-chunked pieces so the transfer spreads across DMA queues.
                ld_chunk = max(1, ko_n // 8)
                for i in range(0, ko_n, ld_chunk):
                    j = min(i + ld_chunk, ko_n)
                    nc.sync.dma_start(xh[:, i:j, :], xhi[:, i:j, m0:m0 + m_block])
                    nc.sync.dma_start(xl[:, i:j, :], xlo[:, i:j, m0:m0 + m_block])

                for nc0 in range(n_nc):
                    c0 = nc0 * n_chunk
                    psums = [
                        pspool.tile([P, MM_N], f32, tag="ps",
                                    name=f"ps_{rep}_{mb}_{nc0}_{i}")
                        for i in range(ms_n * nt_n)
                    ]
                    for ko in range(ko_n):
                        wh = wpool.tile([P, n_chunk], bf16, tag="wh")
                        wl = wpool.tile([P, n_chunk], bf16, tag="wl")
                        nc.sync.dma_start(wh, whi[:, ko, c0:c0 + n_chunk])
                        nc.sync.dma_start(wl, wlo[:, ko, c0:c0 + n_chunk])
                        first = ko == 0
                        last = ko == ko_n - 1
                        for ms in range(ms_n):
                            lh = xh[:, ko, ms * P:(ms + 1) * P]
                            ll = xl[:, ko, ms * P:(ms + 1) * P]
                            # Rotate PSUM banks on every matmul (consecutive
                            # MMs into the same bank stall the PE) while
                            # keeping the stationary operand grouped.
                            for lhs, w_t, st_flag, sp_flag in (
                                (lh, wh, first, False),
                                (lh, wl, False, False),
                                (ll, wh, False, last),
                            ):
                                for nt in range(nt_n):
                                    nc.tensor.matmul(
                                        psums[ms * nt_n + nt],
                                        lhs,
                                        w_t[:, nt * MM_N:(nt + 1) * MM_N],
                                        start=st_flag,
                                        stop=sp_flag,
                                    )
                    # Evict the 8 accumulators for this (mb, nc0) phase.
                    for ms in range(ms_n):
                        for nt in range(nt_n):
                            st = opool.tile([P, MM_N], f32, tag="st")
                            nc.vector.tensor_copy(out=st, in_=psums[ms * nt_n + nt])
                            nc.sync.dma_start(
                                out[m0 + ms * P:m0 + (ms + 1) * P,
                                    c0 + nt * MM_N:c0 + (nt + 1) * MM_N],
                                st,
                            )
    nc.compile()
    return nc


def _build_nc_v2(k, m_core, n, kb_n=4, reps=1):
    """K-blocked variant: full-m x block resident per K-block, W streamed
    exactly once, output accumulated across K-blocks in DRAM via SWDGE
    CCE-add. Total HBM traffic ~2.5x lower than _build_nc."""
    import concourse.mybir as mybir
    import concourse.tile as tile
    from concourse import bacc

    ko_n = k // P            # 128-row k chunks overall
    ko_b = ko_n // kb_n      # k chunks per block
    ms_n = m_core // P       # m strips (psum tiles per chunk)
    nb_n = n // MM_N         # 512-wide n chunks
    assert ms_n <= 8, "PSUM banks exceeded"

    nc = bacc.Bacc(None, target_bir_lowering=False, debug=False)
    xhi = nc.declare_dram_parameter("xhi", [P, ko_n, m_core], mybir.dt.bfloat16, isOutput=False)
    xlo = nc.declare_dram_parameter("xlo", [P, ko_n, m_core], mybir.dt.bfloat16, isOutput=False)
    whi = nc.declare_dram_parameter("whi", [P, ko_n, n], mybir.dt.bfloat16, isOutput=False)
    wlo = nc.declare_dram_parameter("wlo", [P, ko_n, n], mybir.dt.bfloat16, isOutput=False)
    out = nc.declare_dram_parameter("out", [m_core, n], mybir.dt.float32, isOutput=True)

    f32 = mybir.dt.float32
    bf16 = mybir.dt.bfloat16
    add = mybir.AluOpType.add

    with tile.TileContext(nc) as tc:
        with (
            tc.tile_pool(name="xpool", bufs=2 * ko_b + 4) as xpool,
            tc.tile_pool(name="wpool", bufs=8) as wpool,
            tc.tile_pool(name="opool", bufs=4) as opool,
            tc.tile_pool(name="pspool", bufs=8, space="PSUM") as pspool,
        ):
            for rep in range(reps):
                for kb in range(kb_n):
                    k0 = kb * ko_b
                    # Resident x tiles for this K-block: one [P, m_core] tile
                    # per (ko, hi/lo). Spare pool slots let the next block's
                    # first chunks prefetch under this block's tail.
                    xts = []
                    for i in range(ko_b):
                        xh = xpool.tile([P, m_core], bf16, tag="xt",
                                        name=f"xh_{rep}_{kb}_{i}")
                        xl = xpool.tile([P, m_core], bf16, tag="xt",
                                        name=f"xl_{rep}_{kb}_{i}")
                        nc.sync.dma_start(xh, xhi[:, k0 + i, :])
                        nc.sync.dma_start(xl, xlo[:, k0 + i, :])
                        xts.append((xh, xl))
                    for nb in range(nb_n):
                        c0 = nb * MM_N
                        psums = [
                            pspool.tile([P, MM_N], f32, tag="ps",
                                        name=f"ps_{rep}_{kb}_{nb}_{i}")
                            for i in range(ms_n)
                        ]
                        for i in range(ko_b):
                            wh = wpool.tile([P, MM_N], bf16, tag="wh")
                            wl = wpool.tile([P, MM_N], bf16, tag="wl")
                            nc.sync.dma_start(wh, whi[:, k0 + i, c0:c0 + MM_N])
                            nc.sync.dma_start(wl, wlo[:, k0 + i, c0:c0 + MM_N])
                            first = i == 0
                            last = i == ko_b - 1
                            xh, xl = xts[i]
                            # Bank-rotating order: consecutive MMs never hit
                            # the same PSUM bank.
                            for hi_lo, w_t, st_flag, sp_flag in (
                                (0, wh, first, False),
                                (0, wl, False, False),
                                (1, wh, False, last),
                            ):
                                for ms in range(ms_n):
                                    src = xts[i][hi_lo]
                                    nc.tensor.matmul(
                                        psums[ms],
                                        src[:, ms * P:(ms + 1) * P],
                                        w_t,
                                        start=st_flag,
                                        stop=sp_flag,
                                    )
                        for ms in range(ms_n):
                            st = opool.tile([P, MM_N], f32, tag="st")
                            nc.vector.tensor_copy(out=st, in_=psums[ms])
                            dst = out[ms * P:(ms + 1) * P, c0:c0 + MM_N]
                            if kb == 0 and rep == 0:
                                nc.sync.dma_start(dst, st)
                            else:
                                nc.gpsimd.dma_start(dst, st, accum_op=add)
    nc.compile()
    return nc


def _build_nc_v3(k, m_core, n, m_block=512, n_chunk=1024, reps=1):
    """Single-pass bf16 GEMM: y = x16 @ W16^T accumulated in fp32 PSUM.

    The 2e-2 rel-err gate admits plain bf16 operand rounding (~1.5e-3 on this
    K=16384 randn problem), so the hi/lo correction passes of v1/v2 are pure
    overhead: dropping them cuts PE work 3x. Structure mirrors v1: per m-block
    the full-K x strip stays resident in SBUF; W streams once per m-block in
    [128, n_chunk] tiles; ms_n*nt_n PSUM banks accumulate over the whole K
    loop and are evicted once per (m-block, n-chunk) phase."""
    import concourse.mybir as mybir
    import concourse.tile as tile
    from concourse import bacc

    ko_n = k // P               # number of 128-row k chunks
    n_mb = m_core // m_block
    n_nc = n // n_chunk
    ms_n = m_block // P         # m strips per block
    nt_n = n_chunk // MM_N      # n tiles per chunk
    assert ms_n * nt_n <= 8, "PSUM banks exceeded"

    nc = bacc.Bacc(None, target_bir_lowering=False, debug=False)
    xhi = nc.declare_dram_parameter("xhi", [P, ko_n, m_core], mybir.dt.bfloat16, isOutput=False)
    whi = nc.declare_dram_parameter("whi", [P, ko_n, n], mybir.dt.bfloat16, isOutput=False)
    out = nc.declare_dram_parameter("out", [m_core, n], mybir.dt.float32, isOutput=True)

    f32 = mybir.dt.float32
    bf16 = mybir.dt.bfloat16

    with tile.TileContext(nc) as tc:
        with (
            tc.tile_pool(name="xpool", bufs=1) as xpool,
            tc.tile_pool(name="wpool", bufs=6) as wpool,
            tc.tile_pool(name="opool", bufs=4) as opool,
            tc.tile_pool(name="pspool", bufs=8, space="PSUM") as pspool,
        ):
            for rep, mb in ((r_, m_) for r_ in range(reps) for m_ in range(n_mb)):
                m0 = mb * m_block
                xh = xpool.tile([P, ko_n, m_block], bf16, tag="xh")
                ld_chunk = max(1, ko_n // 8)
                for i in range(0, ko_n, ld_chunk):
                    j = min(i + ld_chunk, ko_n)
                    nc.sync.dma_start(xh[:, i:j, :], xhi[:, i:j, m0:m0 + m_block])

                for nc0 in range(n_nc):
                    c0 = nc0 * n_chunk
                    psums = [
                        pspool.tile([P, MM_N], f32, tag="ps",
                                    name=f"ps_{rep}_{mb}_{nc0}_{i}")
                        for i in range(ms_n * nt_n)
                    ]
                    for ko in range(ko_n):
                        wh = wpool.tile([P, n_chunk], bf16, tag="wh")
                        nc.sync.dma_start(wh, whi[:, ko, c0:c0 + n_chunk])
                        first = ko == 0
                        last = ko == ko_n - 1
                        for ms in range(ms_n):
                            lh = xh[:, ko, ms * P:(ms + 1) * P]
                            for nt in range(nt_n):
                                nc.tensor.matmul(
                                    psums[ms * nt_n + nt],
                                    lh,
                                    wh[:, nt * MM_N:(nt + 1) * MM_N],
                                    start=first,
                                    stop=last,
                                )
                    for ms in range(ms_n):
                        for nt in range(nt_n):
                            st = opool.tile([P, MM_N], f32, tag="st")
                            nc.vector.tensor_copy(out=st, in_=psums[ms * nt_n + nt])
                            nc.sync.dma_start(
                                out[m0 + ms * P:m0 + (ms + 1) * P,
                                    c0 + nt * MM_N:c0 + (nt + 1) * MM_N],
                                st,
                            )
    nc.compile()
    return nc


def _build_nc_v4(k, m_core, n, m_block=512, n_chunk=1024, kog=4, wbufs=6,
                 reps=1):
    """v3 + DMA restructuring (the current default):
    - W loaded in [P, kog, n_chunk] slabs (1 MB at kog=4) instead of per-ko
      tiles: fewer, larger DMAs amortize the ~2us per-DMA completion latency.
    - W/x loads alternate between the two HWDGE rings (sync + scalar) so one
      ring's FIFO never serializes the whole stream.
    - PSUM evictions go out on the gpsimd (SWDGE) ring, keeping both HWDGE
      rings free for the W stream and avoiding phase-boundary stalls where
      next-phase W loads would queue behind output stores."""
    import concourse.mybir as mybir
    import concourse.tile as tile
    from concourse import bacc

    ko_n = k // P
    n_mb = m_core // m_block
    n_nc = n // n_chunk
    ms_n = m_block // P
    nt_n = n_chunk // MM_N
    assert ms_n * nt_n <= 8, "PSUM banks exceeded"

    nc = bacc.Bacc(None, target_bir_lowering=False, debug=False)
    xhi = nc.declare_dram_parameter("xhi", [P, ko_n, m_core], mybir.dt.bfloat16, isOutput=False)
    whi = nc.declare_dram_parameter("whi", [P, ko_n, n], mybir.dt.bfloat16, isOutput=False)
    out = nc.declare_dram_parameter("out", [m_core, n], mybir.dt.float32, isOutput=True)

    f32 = mybir.dt.float32
    bf16 = mybir.dt.bfloat16

    with tile.TileContext(nc) as tc:
        with (
            tc.tile_pool(name="xpool", bufs=1) as xpool,
            tc.tile_pool(name="wpool", bufs=wbufs) as wpool,
            tc.tile_pool(name="opool", bufs=4) as opool,
            tc.tile_pool(name="pspool", bufs=8, space="PSUM") as pspool,
        ):
            for rep, mb in ((r_, m_) for r_ in range(reps) for m_ in range(n_mb)):
                m0 = mb * m_block
                xh = xpool.tile([P, ko_n, m_block], bf16, tag="xh")
                ld_chunk = max(1, ko_n // 8)
                for ci, i in enumerate(range(0, ko_n, ld_chunk)):
                    j = min(i + ld_chunk, ko_n)
                    eng = nc.sync if ci % 2 == 0 else nc.scalar
                    eng.dma_start(xh[:, i:j, :], xhi[:, i:j, m0:m0 + m_block])

                for nc0 in range(n_nc):
                    c0 = nc0 * n_chunk
                    psums = [
                        pspool.tile([P, MM_N], f32, tag="ps",
                                    name=f"ps_{rep}_{mb}_{nc0}_{i}")
                        for i in range(ms_n * nt_n)
                    ]
                    for kb in range(0, ko_n, kog):
                        slab = wpool.tile([P, kog, n_chunk], bf16, tag="ws")
                        eng = nc.sync if (kb // kog) % 2 == 0 else nc.scalar
                        eng.dma_start(slab, whi[:, kb:kb + kog, c0:c0 + n_chunk])
                        for i in range(kog):
                            ko = kb + i
                            first, last = ko == 0, ko == ko_n - 1
                            for ms in range(ms_n):
                                lh = xh[:, ko, ms * P:(ms + 1) * P]
                                for nt in range(nt_n):
                                    nc.tensor.matmul(
                                        psums[ms * nt_n + nt], lh,
                                        slab[:, i, nt * MM_N:(nt + 1) * MM_N],
                                        start=first, stop=last)
                    for ms in range(ms_n):
                        for nt in range(nt_n):
                            st = opool.tile([P, MM_N], f32, tag="st")
                            nc.vector.tensor_copy(out=st, in_=psums[ms * nt_n + nt])
                            nc.gpsimd.dma_start(
                                out[m0 + ms * P:m0 + (ms + 1) * P,
                                    c0 + nt * MM_N:c0 + (nt + 1) * MM_N],
                                st)
    nc.compile()
    return nc


def _build_nc_v5(k, m_core, n, m_block=512, n_chunk=1024, kog=4, wbufs=4,
                 xch=8, reps=1):
    """v4 but the x strip is built from per-xch-ko chunk tiles in a deep pool
    (bufs = chunks-per-block + 2), so the next m-block's x prefetches under
    the current block's last phase instead of stalling ~80us at the
    boundary."""
    import concourse.mybir as mybir
    import concourse.tile as tile
    from concourse import bacc

    ko_n = k // P
    n_mb = m_core // m_block
    n_nc = n // n_chunk
    ms_n = m_block // P
    nt_n = n_chunk // MM_N
    nch = ko_n // xch
    assert ms_n * nt_n <= 8, "PSUM banks exceeded"

    nc = bacc.Bacc(None, target_bir_lowering=False, debug=False)
    xhi = nc.declare_dram_parameter("xhi", [P, ko_n, m_core], mybir.dt.bfloat16, isOutput=False)
    whi = nc.declare_dram_parameter("whi", [P, ko_n, n], mybir.dt.bfloat16, isOutput=False)
    out = nc.declare_dram_parameter("out", [m_core, n], mybir.dt.float32, isOutput=True)

    f32 = mybir.dt.float32
    bf16 = mybir.dt.bfloat16

    with tile.TileContext(nc) as tc:
        with (
            tc.tile_pool(name="xpool", bufs=nch + 2) as xpool,
            tc.tile_pool(name="wpool", bufs=wbufs) as wpool,
            tc.tile_pool(name="opool", bufs=4) as opool,
            tc.tile_pool(name="pspool", bufs=8, space="PSUM") as pspool,
        ):
            for rep, mb in ((r_, m_) for r_ in range(reps) for m_ in range(n_mb)):
                m0 = mb * m_block
                xts = []
                for ci in range(nch):
                    xt = xpool.tile([P, xch, m_block], bf16, tag="xc",
                                    name=f"xc_{rep}_{mb}_{ci}")
                    eng = nc.sync if ci % 2 == 0 else nc.scalar
                    eng.dma_start(xt, xhi[:, ci * xch:(ci + 1) * xch,
                                          m0:m0 + m_block])
                    xts.append(xt)
                for nc0 in range(n_nc):
                    c0 = nc0 * n_chunk
                    psums = [
                        pspool.tile([P, MM_N], f32, tag="ps",
                                    name=f"ps_{rep}_{mb}_{nc0}_{i}")
                        for i in range(ms_n * nt_n)
                    ]
                    for kb in range(0, ko_n, kog):
                        slab = wpool.tile([P, kog, n_chunk], bf16, tag="ws")
                        eng = nc.sync if (kb // kog) % 2 == 0 else nc.scalar
                        eng.dma_start(slab, whi[:, kb:kb + kog, c0:c0 + n_chunk])
                        for i in range(kog):
                            ko = kb + i
                            first, last = ko == 0, ko == ko_n - 1
                            xt = xts[ko // xch]
                            for ms in range(ms_n):
                                lh = xt[:, ko % xch, ms * P:(ms + 1) * P]
                                for nt in range(nt_n):
                                    nc.tensor.matmul(
                                        psums[ms * nt_n + nt], lh,
                                        slab[:, i, nt * MM_N:(nt + 1) * MM_N],
                                        start=first, stop=last)
                    for ms in range(ms_n):
                        for nt in range(nt_n):
                            st = opool.tile([P, MM_N], f32, tag="st")
                            nc.vector.tensor_copy(out=st, in_=psums[ms * nt_n + nt])
                            nc.gpsimd.dma_start(
                                out[m0 + ms * P:m0 + (ms + 1) * P,
                                    c0 + nt * MM_N:c0 + (nt + 1) * MM_N],
                                st)
    nc.compile()
    return nc


def _build_nc_v6(k, m_core, n, reps=1):
    """v5 with r=4 stationary reuse (m_block=256, n_chunk=2048): halves the
    residual LDWEIGHTS exposure at the cost of streaming W 4x (537MB/core)."""
    return _build_nc_v5(k, m_core, n, m_block=256, n_chunk=2048, kog=4,
                        wbufs=4, xch=16, reps=reps)


_BUILDERS = {"v1": _build_nc, "v2": _build_nc_v2, "v3": _build_nc_v3,
             "v4": _build_nc_v4, "v5": _build_nc_v5, "v6": _build_nc_v6}


def _variant():
    return os.environ.get("KERNEL_VARIANT", "v5")


def _get_nc(k, m_core, n, **kw):
    variant = _variant()
    key = (variant, k, m_core, n, tuple(sorted(kw.items())))
    if key not in _BUILD_CACHE:
        _BUILD_CACHE[key] = _BUILDERS[variant](k, m_core, n, **kw)
    return _BUILD_CACHE[key]


def _split_bf16(a_f32):
    """Exact split a = hi + lo with hi, lo bf16 (a contiguous fp32 array)."""
    hi = a_f32.astype(BF16)
    lo = (a_f32 - hi.astype(np.float32)).astype(BF16)
    return hi, lo


def _to_pkm(a, ko_n):
    """[rows, k] fp32 -> contiguous [P, ko_n, rows] (k = ko*128 + p)."""
    rows = a.shape[0]
    return np.ascontiguousarray(a.reshape(rows, ko_n, P).transpose(2, 1, 0))


def _make_runner(nc):
    """Build the sharded PJRT executor for `nc` across the 8 cores.

    Mirrors concourse.bass2jax.run_bass_via_pjrt, but returns a reusable
    closure so repeated calls share one jit cache and inputs can stay
    device-resident for benchmarking.
    """
    import jax
    import concourse.mybir as mybir
    from concourse import bass2jax
    from jax.experimental.shard_map import shard_map
    from jax.sharding import Mesh, NamedSharding, PartitionSpec

    bass2jax.install_neuronx_cc_hook()

    partition_name = nc.partition_id_tensor.name if nc.partition_id_tensor else None
    assert nc.dbg_addr is None

    in_names, out_names, out_avals = [], [], []
    for alloc in nc.m.functions[0].allocations:
        if not isinstance(alloc, mybir.MemoryLocationSet):
            continue
        name = alloc.memorylocations[0].name
        if alloc.kind == "ExternalInput":
            if name != partition_name:
                in_names.append(name)
        elif alloc.kind == "ExternalOutput":
            out_names.append(name)
            out_avals.append(
                jax.core.ShapedArray(tuple(alloc.tensor_shape), mybir.dt.np(alloc.dtype))
            )
    n_params = len(in_names)
    n_outs = len(out_avals)
    all_in_names = tuple(in_names) + tuple(out_names)
    if partition_name is not None:
        all_in_names = all_in_names + (partition_name,)
    donate = tuple(range(n_params, n_params + n_outs))

    def _body(*args):
        operands = list(args)
        if partition_name is not None:
            operands.append(bass2jax.partition_id_tensor())
        outs = bass2jax._bass_exec_p.bind(
            *operands,
            out_avals=tuple(out_avals),
            in_names=all_in_names,
            out_names=tuple(out_names),
            lowering_input_output_aliases=(),
            sim_require_finite=True,
            sim_require_nnan=True,
            nc=nc,
        )
        return tuple(outs)

    devices = jax.devices()[:N_CORES]
    assert len(devices) == N_CORES
    mesh = Mesh(np.asarray(devices), ("core",))
    spec = PartitionSpec("core")
    sharded = jax.jit(
        shard_map(
            _body,
            mesh=mesh,
            in_specs=(spec,) * (n_params + n_outs),
            out_specs=(spec,) * n_outs,
            check_rep=False,
        ),
        donate_argnums=donate,
        keep_unused=True,
    )
    sharding = NamedSharding(mesh, spec)
    return {
        "sharded": sharded,
        "sharding": sharding,
        "in_names": in_names,
        "out_names": out_names,
        "out_avals": out_avals,
        "n_params": n_params,
        "n_outs": n_outs,
    }


def _get_runner(nc):
    key = id(nc)
    if key not in _RUNNER_CACHE:
        _RUNNER_CACHE[key] = _make_runner(nc)
    return _RUNNER_CACHE[key]


def _run(nc, in_maps):
    """Execute the kernel across 8 cores; returns per-core output dicts."""
    import numpy as np

    r = _get_runner(nc)
    n_cores = len(in_maps)
    concat_in = [
        np.concatenate([np.asarray(m[name]) for m in in_maps], axis=0)
        for name in r["in_names"]
    ]
    concat_zeros = [
        np.zeros((n_cores * a.shape[0], *a.shape[1:]), a.dtype) for a in r["out_avals"]
    ]
    out_arrs = r["sharded"](*concat_in, *concat_zeros)
    return [
        {
            name: np.asarray(out_arrs[i]).reshape(n_cores, *r["out_avals"][i].shape)[c]
            for i, name in enumerate(r["out_names"])
        }
        for c in range(n_cores)
    ]


def _bench(in_maps, k, m_core, n, reps):
    """Measure steady-state per-GEMM time: wall-clock of the reps=N program
    minus the reps=1 program, divided by N-1, with the two programs timed in
    interleaved pairs (median of pairs) so slow drift in the ~80ms fixed
    dispatch overhead cancels instead of polluting the difference.
    Sets LAST_EXEC_NS."""
    global LAST_EXEC_NS
    import time

    import jax
    import jax.numpy as jnp
    import numpy as np

    runners = {}
    dev_in = None
    for r_reps in (1, reps):
        nc = _get_nc(k, m_core, n) if r_reps == 1 else _get_nc(k, m_core, n, reps=r_reps)
        r = _get_runner(nc)
        runners[r_reps] = r
        if dev_in is None:
            concat_in = [
                np.concatenate([np.asarray(m[name]) for m in in_maps], axis=0)
                for name in r["in_names"]
            ]
            dev_in = [jax.device_put(a, r["sharding"]) for a in concat_in]
            jax.block_until_ready(dev_in)

    def _zeros(r):
        zs = [
            jax.jit(lambda a=a: jnp.zeros(a.shape, a.dtype),
                    out_shardings=r["sharding"])()
            for a in r["out_avals"]
        ]
        jax.block_until_ready(zs)
        return zs

    def _timed(r):
        zs = _zeros(r)
        t0 = time.perf_counter()
        o = r["sharded"](*dev_in, *zs)
        jax.block_until_ready(o)
        return time.perf_counter() - t0

    for r in runners.values():  # compile + warmup
        _timed(r)

    diffs = []
    for i in range(5):
        try:
            t1 = _timed(runners[1])
            tn = _timed(runners[reps])
        except Exception as e:  # keep earlier pairs if a run dies mid-bench
            print(f"[bench] pair {i} failed: {e!r}")
            break
        d = (tn - t1) / (reps - 1)
        diffs.append(d)
        print(f"[bench] pair {i}: t1={t1 * 1e3:.2f}ms tN={tn * 1e3:.2f}ms "
              f"-> per-GEMM {d * 1e3:.3f}ms")
    if not diffs:
        return
    # Drift occasionally makes a pair negative; median over positive diffs
    # unless noise swamped every pair.
    pos = [d for d in diffs if d > 0]
    per_iter = float(np.median(pos if pos else diffs))
    LAST_EXEC_NS = int(per_iter * 1e9)
    print(f"[bench] per-GEMM: {per_iter * 1e3:.3f} ms (median of pairs)")


def kernel(input_, weight, bias):
    global LAST_RESULTS

    input_ = np.asarray(input_, dtype=np.float32)
    weight = np.asarray(weight, dtype=np.float32)
    bias = np.asarray(bias, dtype=np.float32)

    seq, batch, k = input_.shape
    n = weight.shape[0]
    m_full = seq * batch
    m_core = m_full // N_CORES
    ko_n = k // P

    nc = _get_nc(k, m_core, n)

    single_pass = _variant() in ("v3", "v4", "v5", "v6")
    x2 = input_.reshape(m_full, k)
    wT = _to_pkm(weight, ko_n)                  # [P, ko, n] fp32
    if single_pass:
        whi, wlo = wT.astype(BF16), None
    else:
        whi, wlo = _split_bf16(wT)
    del wT

    in_maps = []
    for c in range(N_CORES):
        xcT = _to_pkm(x2[c * m_core:(c + 1) * m_core], ko_n)  # [P, ko, m_core]
        if single_pass:
            in_maps.append({"xhi": xcT.astype(BF16), "whi": whi})
        else:
            xh, xl = _split_bf16(xcT)
            in_maps.append({"xhi": xh, "xlo": xl, "whi": whi, "wlo": wlo})
        del xcT

    results = _run(nc, in_maps)
    LAST_RESULTS = results

    bench_reps = int(os.environ.get("KERNEL_BENCH", "0"))
    if bench_reps > 1:
        try:
            _bench(in_maps, k, m_core, n, bench_reps)
        except Exception as e:
            # Never let benchmarking failures mask a correct kernel result.
            print(f"[bench] failed: {e!r}")

    out = np.concatenate([results[c]["out"] for c in range(N_CORES)], axis=0)
    out = out.reshape(seq, batch, n)
    if bias.any():
        out = out + bias
    return out

